# revision 24
# baseline (speedup 1.0000x reference)
"""Trainium2 Bass kernel for nn_MultiHeadSelfAttention_67559835566279.

Module: x -> [sep_conv(q, stride1), sep_conv(kv, stride2)] -> 3-head attention
        -> output projection.  B=8 samples, data-parallel: one sample per core.

Per-core layout (all "transposed" activations keep channels on SBUF
partitions so every matmul contracts along partitions):
  - host pre-pads/transposes x to xT_pad [C=192, 58*58] bf16 (zero border)
  - depthwise convs as per-tap diag-matmuls on PE (+ a few taps on DVE),
    pointwise convs as matmuls contracting C
  - attention per head: S^T[j,i] = k-slices.T @ q, exp on ACT (PSUM->SBUF
    bf16), PV accumulates o^T[65, i]; a ones-column in V yields softmax
    denominators in row 64 for free
  - normalization: fast approx reciprocal of the denominator row, partition
    broadcast by DMA, fused into the PSUM evacuation multiply
  - projection: o^T tiles stationary; ones-row adds out_b; result DMAs
    straight from PSUM to DRAM

Scheduling structure (v2):
  - PSUM pools span the whole kernel (2 tags: "s" 2x2 banks, "o" 4x1 bank)
    so conv and attention phases hand off banks per-tile instead of
    serializing on pool release
  - i-chunks run in ascending order and interleave with the dwq windows
    they depend on; projection is inline per chunk
  - heads 0/1 scores matmuls are emitted adjacently with disjoint PE row
    groups (contraction rows 0-63 vs 64-127) so hardware runs them
    concurrently
"""

import numpy as np
import ml_dtypes

import concourse.bass as bass
import concourse.tile as tile
from concourse import bacc
from concourse import mybir
from contextlib import ExitStack

F32 = mybir.dt.float32
BF16 = mybir.dt.bfloat16
AOP = mybir.AluOpType
AF = mybir.ActivationFunctionType

B = 8
C = 192
H = W = 56
HP = 58                      # padded spatial
NPAD = HP * HP               # 3364
N = H * W                    # 3136
NKV = 28 * 28                # 784
HEADS = 3
DH = 64
EPS = 1e-5
SCALE = np.float32(64.0) ** np.float32(-0.5)

CC = [(0, 128), (128, 64)]   # channel chunks (offset, size)
DC = [(0, 128), (128, 64)]   # inner-dim chunks
TAPS = [(kh, kw) for kh in range(3) for kw in range(3)]
J_SZ = 112                   # 784 = 7 * 112

# attention runs in 512-col i-blocks; heads 0/1 share one [112,1024] score
# tile (bank-aligned halves), head 2 processes block PAIRS in one tile
I_BLOCKS = [(b * 512, min(512, N - b * 512)) for b in range((N + 511) // 512)]
# dwq windows (448 cols) emitted before the block that needs them
WIN_BEFORE_BLOCK = {0: [0, 1, 2], 1: [3], 2: [4], 3: [5], 4: [6], 5: [], 6: []}

KNOBS = {
    "dve_taps": 4,        # dwq taps on DVE (rest on PE)
    "gp_taps": 0,         # dwq taps on GPSIMD
    "conv_evac": "act",   # act | dve | mix
    "bcast": "dram",      # sbuf | dram  (reciprocal broadcast route)
    "proj_evac": "dve",   # dve | act
    "s_slots": 2,
    "o_slots": 3,
    "c_slots": 1,
    "e_bufs": 8,
    "recip": "exact",     # approx is silently wrong on this HW
}


def _as_f32(a):
    return np.ascontiguousarray(np.asarray(a, dtype=np.float32))


def build_nc(repeat=1, **knobs):
    KNOBS.update(knobs)
    nc = bacc.Bacc("TRN2", target_bir_lowering=False, debug=False, num_devices=B)

    din = {}
    def dram_in(name, shape, dtype):
        din[name] = nc.dram_tensor(name, shape, dtype, kind="ExternalInput").ap()
        return din[name]

    dram_in("xtp", [C, NPAD], BF16)
    dram_in("dwq1", [128, 9], F32)        # per-partition tap scalars (DVE path)
    dram_in("dwq2", [64, 9], F32)
    dram_in("dwkv1c", [128, 9], F32)
    dram_in("dwkv2c", [64, 9], F32)
    dram_in("qd1", [128, 9 * 128], BF16)  # diag tap matrices (PE path)
    dram_in("qd2", [64, 9 * 64], BF16)
    dram_in("kvd1", [128, 9 * 128], BF16)
    dram_in("kvd2", [64, 9 * 64], BF16)
    dram_in("pwq1", [128, 192], BF16)
    dram_in("pwq2", [64, 192], BF16)
    dram_in("pwkv1", [128, 384], BF16)
    dram_in("pwkv2", [65, 384], BF16)     # row 64: [0:192]=0, [192:384]=b_v
    dram_in("waug1", [128, 192], BF16)
    dram_in("waug2", [65, 192], BF16)     # row 64 = out_b
    dram_in("bias1", [128, 2], F32)       # col0=b_q col1=b_k
    dram_in("bias2", [64, 2], F32)

    out = nc.dram_tensor("out", [N, C], F32, kind="ExternalOutput").ap()
    dbg = {}
    if KNOBS.get("dbg_dump"):
        for nm, shp in [("d_qt", [C, N]), ("d_kt", [C, NKV]),
                        ("d_qt1b", [64, N]), ("d_kt1b", [64, NKV]),
                        ("d_v", [J_SZ, 7 * 195]),
                        ("d_oTA", [128, N]), ("d_oTB", [65, N])]:
            dbg[nm] = nc.dram_tensor(nm, shp, BF16, kind="ExternalOutput").ap()

    with tile.TileContext(nc) as tc:
        for rep in range(repeat):
            with ExitStack() as ctx:
                _emit(ctx, tc, din, out, suffix=f"_r{rep}" if repeat > 1 else "",
                      dbg=dbg)
    nc.compile()
    return nc


def _emit(ctx, tc, din, out, suffix="", dbg=None):
    nc = tc.nc
    def _nm(s):
        return s + suffix

    consts = ctx.enter_context(tc.tile_pool(name="consts", bufs=1))
    acts = ctx.enter_context(tc.tile_pool(name="acts", bufs=1))
    # PSUM pools live for the whole kernel: conv + attention share slots.
    spool = ctx.enter_context(
        tc.tile_pool(name="ps_s", bufs=KNOBS["s_slots"], space="PSUM"))
    opool = ctx.enter_context(
        tc.tile_pool(name="ps_o", bufs=KNOBS["o_slots"], space="PSUM"))
    cpool = ctx.enter_context(
        tc.tile_pool(name="ps_c", bufs=KNOBS["c_slots"], space="PSUM"))
    epool = ctx.enter_context(tc.tile_pool(name="expS", bufs=KNOBS["e_bufs"]))
    rpool = ctx.enter_context(tc.tile_pool(name="recip", bufs=4))
    rbpool = ctx.enter_context(tc.tile_pool(name="rbcast", bufs=4))
    if KNOBS["bcast"] == "dram":
        rdpool = ctx.enter_context(tc.tile_pool(name="rdram", bufs=2, space="DRAM"))
    osbpool = ctx.enter_context(tc.tile_pool(name="outsb", bufs=4))

    # ---- static loads -------------------------------------------------
    def load(name):
        src = din[name]
        t = consts.tile(list(src.shape), src.dtype, tag=name)
        nc.sync.dma_start(out=t[:, :], in_=src[:, :])
        return t

    xtp1 = consts.tile([128, NPAD], BF16, tag="xtp1", name=_nm("xtp1"))
    xtp2 = consts.tile([64, NPAD], BF16, tag="xtp2", name=_nm("xtp2"))
    for q in range(4):
        f0, f1 = q * (NPAD // 4), (q + 1) * (NPAD // 4) if q < 3 else NPAD
        nc.sync.dma_start(out=xtp1[:, f0:f1], in_=din["xtp"][0:128, f0:f1])
        nc.sync.dma_start(out=xtp2[:, f0:f1], in_=din["xtp"][128:192, f0:f1])
    xv = [xtp1[:, :].rearrange("p (h w) -> p h w", h=HP, w=HP),
          xtp2[:, :].rearrange("p (h w) -> p h w", h=HP, w=HP)]

    kvd = [load("kvd1"), load("kvd2")]
    dwq_w = [load("dwq1"), load("dwq2")]
    dwkv_w = [load("dwkv1c"), load("dwkv2c")]
    pwkv = [load("pwkv1"), load("pwkv2")]
    qd = [load("qd1"), load("qd2")]
    pwq = [load("pwq1"), load("pwq2")]
    waug = [load("waug1"), load("waug2")]
    bias = [load("bias1"), load("bias2")]

    # ---- activations (persistent SBUF) --------------------------------
    dwq_sb = [acts.tile([128, N], BF16, tag="dwq1s", name=_nm("dwq1s")),
              acts.tile([64, N], BF16, tag="dwq2s", name=_nm("dwq2s"))]
    dwkv_sb = [acts.tile([128, NKV], BF16, tag="dwkv1s", name=_nm("dwkv1s")),
               acts.tile([65, NKV], BF16, tag="dwkv2s", name=_nm("dwkv2s"))]  # row 64 = ones
    qT = [acts.tile([128, N], BF16, tag="qt1", name=_nm("qt1")),
          acts.tile([64, N], BF16, tag="qt2", name=_nm("qt2"))]
    kT = [acts.tile([128, NKV], BF16, tag="kt1", name=_nm("kt1")),
          acts.tile([64, NKV], BF16, tag="kt2", name=_nm("kt2"))]
    vsb = [acts.tile([J_SZ, 3 * 65], BF16, tag=f"v{j}", name=_nm(f"v{j}"))
           for j in range(7)]
    oTA = acts.tile([128, N], BF16, tag="oTA", name=_nm("oTA"))
    oTB = acts.tile([65, N], BF16, tag="oTB", name=_nm("oTB"))   # row 64 = ones
    # head-1 k/q relocated to base partition 0: keeps every matmul at
    # tile_position (0,0) (mixing (0,0)/(64,0) stationaries hangs the device)
    qt1b = acts.tile([64, N], BF16, tag="qt1b", name=_nm("qt1b"))
    kt1b = acts.tile([64, NKV], BF16, tag="kt1b", name=_nm("kt1b"))

    nc.gpsimd.memset(dwkv_sb[1][64:65, :], 1.0)
    nc.gpsimd.memset(oTB[64:65, :], 1.0)
    for j in range(7):
        nc.gpsimd.memset(
            vsb[j][:, :].rearrange("p (h d) -> p h d", h=3)[:, :, 64:65], 1.0)

    _evac_ctr = [0]
    def conv_evac(dst, src_ps, bias_ap=None, accum=False):
        if accum:
            nc.vector.tensor_tensor(out=dst, in0=src_ps, in1=dst, op=AOP.add)
            return
        mode = KNOBS["conv_evac"]
        if mode == "mix":
            mode = "act" if _evac_ctr[0] % 2 == 0 else "dve"
            _evac_ctr[0] += 1
        if mode == "act":
            if bias_ap is None:
                nc.scalar.copy(dst, src_ps)
            else:
                nc.scalar.activation(out=dst, in_=src_ps,
                                     func=AF.Identity, bias=bias_ap, scale=1.0)
        else:
            if bias_ap is None:
                nc.vector.tensor_copy(dst, src_ps)
            else:
                nc.vector.tensor_scalar(out=dst, in0=src_ps, scalar1=bias_ap,
                                        scalar2=None, op0=AOP.add)

    # =========== kv branch (tag "s" PSUM slots) ========================
    for ci, (c0, csz) in enumerate(CC):
        for j0, jn in [(0, 448), (448, 336)]:   # h' rows 0:16, 16:28
            h0 = (j0 // 28)
            ps = spool.tile([csz, 448], F32, tag="s", name=_nm(f"dwkv{ci}_{j0}"))
            for t, (kh, kw) in enumerate(TAPS):
                hs = 2 * h0 + kh + 1
                ws = kw + 1
                rhs = xv[ci][0:csz,
                             hs: min(hs + 2 * (jn // 28), HP): 2,
                             ws: min(ws + 56, HP): 2]
                nc.tensor.matmul(out=ps[0:csz, 0:jn],
                                 lhsT=kvd[ci][0:csz, t * csz:(t + 1) * csz],
                                 rhs=rhs, start=(t == 0), stop=(t == 8))
            conv_evac(dwkv_sb[ci][0:csz, j0:j0 + jn], ps[0:csz, 0:jn])

    for di, (d0, dsz) in enumerate(DC):       # pointwise k (transposed out)
        for j0, jn in [(0, 448), (448, 336)]:
            ps = spool.tile([dsz, 448], F32, tag="s", name=_nm(f"pwk{di}_{j0}"))
            for ci, (c0, csz) in enumerate(CC):
                nc.tensor.matmul(out=ps[0:dsz, 0:jn],
                                 lhsT=pwkv[ci][0:csz, d0:d0 + dsz],
                                 rhs=dwkv_sb[ci][0:csz, j0:j0 + jn],
                                 start=(ci == 0), stop=(ci == 1))
            conv_evac(kT[di][0:dsz, j0:j0 + jn], ps[0:dsz, 0:jn],
                      bias_ap=bias[di][0:dsz, 1:2])
    nc.sync.dma_start(out=kt1b[0:64, :], in_=kT[0][64:128, :])

    for j in range(7):                        # pointwise v (natural out)
        ps = spool.tile([J_SZ, 192], F32, tag="s", name=_nm(f"pwv{j}"))
        js = slice(j * J_SZ, (j + 1) * J_SZ)
        nc.tensor.matmul(out=ps[:, :], lhsT=dwkv_sb[0][0:128, js],
                         rhs=pwkv[0][0:128, 192:384], start=True, stop=False)
        nc.tensor.matmul(out=ps[:, :], lhsT=dwkv_sb[1][0:65, js],
                         rhs=pwkv[1][0:65, 192:384], start=False, stop=True)
        conv_evac(
            vsb[j][:, :].rearrange("p (h d) -> p h d", h=3)[:, :, 0:64],
            ps[:, :].rearrange("p (h d) -> p h d", h=3))

    # =========== q branch: one 448-col window ==========================
    n_dve = KNOBS["dve_taps"]
    n_gp = KNOBS["gp_taps"]

    def emit_dwq_window(w):
        w0 = w * 448
        h0 = w0 // W
        for ci, (c0, csz) in enumerate(CC):
            dst3 = dwq_sb[ci][0:csz, w0:w0 + 448].rearrange(
                "p (h w) -> p h w", h=8, w=56)
            for t in range(n_dve + n_gp):
                eng = nc.vector if t < n_dve else nc.gpsimd
                kh, kw = TAPS[t]
                srcv = xv[ci][0:csz, h0 + kh:h0 + kh + 8, kw:kw + 56]
                if t == 0:
                    eng.tensor_scalar(
                        out=dst3, in0=srcv, scalar1=dwq_w[ci][0:csz, 0:1],
                        scalar2=None, op0=AOP.mult)
                else:
                    eng.scalar_tensor_tensor(
                        out=dst3, in0=srcv, scalar=dwq_w[ci][0:csz, t:t + 1],
                        in1=dst3, op0=AOP.mult, op1=AOP.add)
            if n_dve + n_gp < 9:
                ps = cpool.tile([csz, 448], F32, tag="c", name=_nm(f"dwq{ci}w{w}"))
                for t in range(n_dve + n_gp, 9):
                    kh, kw = TAPS[t]
                    rhs = xv[ci][0:csz, h0 + kh:h0 + kh + 8, kw:kw + 56]
                    nc.tensor.matmul(out=ps[0:csz, 0:448],
                                     lhsT=qd[ci][0:csz, t * csz:(t + 1) * csz],
                                     rhs=rhs, start=(t == n_dve + n_gp), stop=(t == 8))
                conv_evac(dwq_sb[ci][0:csz, w0:w0 + 448], ps[0:csz, 0:448],
                          accum=(n_dve + n_gp > 0))
        for di, (d0, dsz) in enumerate(DC):
            ps = cpool.tile([dsz, 448], F32, tag="c", name=_nm(f"pwq{di}w{w}"))
            for ci, (c0, csz) in enumerate(CC):
                nc.tensor.matmul(out=ps[0:dsz, 0:448],
                                 lhsT=pwq[ci][0:csz, d0:d0 + dsz],
                                 rhs=dwq_sb[ci][0:csz, w0:w0 + 448],
                                 start=(ci == 0), stop=(ci == 1))
            conv_evac(qT[di][0:dsz, w0:w0 + 448], ps[0:dsz, 0:448],
                      bias_ap=bias[di][0:dsz, 0:1])
        nc.sync.dma_start(out=qt1b[0:64, w0:w0 + 448],
                          in_=qT[0][64:128, w0:w0 + 448])

    # =========== attention =============================================
    def head_kq(h):
        if h == 0:
            return kT[0][0:64], qT[0][0:64]
        if h == 1:
            return kt1b[0:64], qt1b[0:64]
        return kT[1][0:64], qT[1][0:64]

    def head_dst(h):
        return oTA[0:64] if h == 0 else (oTA[64:128] if h == 1 else oTB[0:64])

    def norm_evac(h, i0, gsz, ot, key):
        """o_psum [65, gsz] (row 64 = denom) -> normalized bf16 rows in oT*."""
        dst = head_dst(h)
        rec = rpool.tile([1, 512], F32, tag="r", name=_nm(f"r{key}"))
        if KNOBS["recip"] == "approx":
            nc.vector.reciprocal_approx_fast(rec[:, 0:gsz], ot[64:65, 0:gsz])
        else:
            nc.vector.reciprocal(rec[:, 0:gsz], ot[64:65, 0:gsz])
        rb = rbpool.tile([64, 512], F32, tag="rb", name=_nm(f"rb{key}"))
        if KNOBS["bcast"] == "sbuf":
            nc.gpsimd.dma_start(out=rb[:, 0:gsz],
                                in_=rec[0:1, 0:gsz].to_broadcast((64, gsz)))
        else:
            rd = rdpool.tile([1, 512], F32, tag="rd", name=_nm(f"rd{key}"))
            nc.sync.dma_start(out=rd[:, 0:gsz], in_=rec[:, 0:gsz])
            nc.gpsimd.dma_start(out=rb[:, 0:gsz],
                                in_=rd[0:1, 0:gsz].to_broadcast((64, gsz)))
        nc.vector.tensor_tensor(out=dst[:, i0:i0 + gsz],
                                in0=ot[0:64, 0:gsz], in1=rb[:, 0:gsz],
                                op=AOP.mult)

    def h01_block(b, i0, isz):
        """Heads 0+1 for one 512-col block; shared score tile, h0 in cols
        [0:isz] (PE rows 0-63), h1 in cols [isz:2*isz] (rows 64-127)."""
        o0 = opool.tile([65, 512], F32, tag="o", name=_nm(f"o{b}h0"))
        o1 = opool.tile([65, 512], F32, tag="o", name=_nm(f"o{b}h1"))
        for j in range(7):
            s01 = spool.tile([112, 1024], F32, tag="s", name=_nm(f"s{b}j{j}"))
            for h in (0, 1):
                k_h, q_h = head_kq(h)
                nc.tensor.matmul(out=s01[:, h * isz:(h + 1) * isz],
                                 lhsT=k_h[:, j * J_SZ:(j + 1) * J_SZ],
                                 rhs=q_h[:, i0:i0 + isz],
                                 start=True, stop=True)
            es = epool.tile([112, 1024], BF16, tag="e", name=_nm(f"e{b}j{j}"))
            nc.scalar.activation(out=es[:, 0:2 * isz], in_=s01[:, 0:2 * isz],
                                 func=AF.Exp)
            nc.tensor.matmul(out=o0[:, 0:isz], lhsT=vsb[j][:, 0:65],
                             rhs=es[:, 0:isz], start=(j == 0), stop=(j == 6))
            nc.tensor.matmul(out=o1[:, 0:isz], lhsT=vsb[j][:, 65:130],
                             rhs=es[:, isz:2 * isz], start=(j == 0), stop=(j == 6))
        norm_evac(0, i0, isz, o0, f"b{b}h0")
        norm_evac(1, i0, isz, o1, f"b{b}h1")

    def h2_pair(blocks):
        """Head 2 for one or two 512-col blocks in a single score tile."""
        o2 = [opool.tile([65, 512], F32, tag="o", name=_nm(f"o{b}h2"))
              for b, _, _ in blocks]
        k_h, q_h = head_kq(2)
        for j in range(7):
            s2 = spool.tile([112, 1024], F32, tag="s",
                            name=_nm(f"s2p{blocks[0][0]}j{j}"))
            for bi, (b, i0, isz) in enumerate(blocks):
                nc.tensor.matmul(out=s2[:, bi * 512:bi * 512 + isz],
                                 lhsT=k_h[:, j * J_SZ:(j + 1) * J_SZ],
                                 rhs=q_h[:, i0:i0 + isz],
                                 start=True, stop=True)
            es = epool.tile([112, 1024], BF16, tag="e",
                            name=_nm(f"e2p{blocks[0][0]}j{j}"))
            hi = blocks[-1][0] - blocks[0][0]
            lim = hi * 512 + blocks[-1][2]
            nc.scalar.activation(out=es[:, 0:lim], in_=s2[:, 0:lim], func=AF.Exp)
            for bi, (b, i0, isz) in enumerate(blocks):
                nc.tensor.matmul(out=o2[bi][:, 0:isz], lhsT=vsb[j][:, 130:195],
                                 rhs=es[:, bi * 512:bi * 512 + isz],
                                 start=(j == 0), stop=(j == 6))
        for bi, (b, i0, isz) in enumerate(blocks):
            norm_evac(2, i0, isz, o2[bi], f"b{b}h2")

    def proj_range(n_lo, n_hi):
        for n0 in range(n_lo, n_hi, 128):
            nsz = min(128, n_hi - n0)
            pr = opool.tile([128, 192], F32, tag="o", name=_nm(f"pr{n0}"))
            nc.tensor.matmul(out=pr[0:nsz, :], lhsT=oTA[:, n0:n0 + nsz],
                             rhs=waug[0][:, :], start=True, stop=False)
            nc.tensor.matmul(out=pr[0:nsz, :], lhsT=oTB[0:65, n0:n0 + nsz],
                             rhs=waug[1][0:65, :], start=False, stop=True)
            osb = osbpool.tile([128, 192], F32, tag="osb", name=_nm(f"osb{n0}"))
            if KNOBS["proj_evac"] == "act":
                nc.scalar.copy(osb[0:nsz, :], pr[0:nsz, :])
            else:
                nc.vector.tensor_copy(osb[0:nsz, :], pr[0:nsz, :])
            nc.sync.dma_start(out=out[n0:n0 + nsz, :], in_=osb[0:nsz, :])

    # =========== interleaved emission ==================================
    pend = []
    for b, (i0, isz) in enumerate(I_BLOCKS):
        for w in WIN_BEFORE_BLOCK[b]:
            emit_dwq_window(w)
        h01_block(b, i0, isz)
        pend.append((b, i0, isz))
        if len(pend) == 2 or b == len(I_BLOCKS) - 1:
            h2_pair(pend)
            lo = pend[0][1]
            hi = pend[-1][1] + pend[-1][2]
            proj_range(lo, hi)
            pend = []

    if dbg:
        nc.gpsimd.dma_start(out=dbg["d_qt"][0:128, :], in_=qT[0][0:128, :])
        nc.gpsimd.dma_start(out=dbg["d_qt"][128:192, :], in_=qT[1][0:64, :])
        nc.gpsimd.dma_start(out=dbg["d_kt"][0:128, :], in_=kT[0][0:128, :])
        nc.gpsimd.dma_start(out=dbg["d_kt"][128:192, :], in_=kT[1][0:64, :])
        nc.gpsimd.dma_start(out=dbg["d_qt1b"][:, :], in_=qt1b[:, :])
        nc.gpsimd.dma_start(out=dbg["d_kt1b"][:, :], in_=kt1b[:, :])
        for j in range(7):
            nc.gpsimd.dma_start(out=dbg["d_v"][:, j * 195:(j + 1) * 195],
                                in_=vsb[j][:, :])
        nc.gpsimd.dma_start(out=dbg["d_oTA"][:, :], in_=oTA[:, :])
        nc.gpsimd.dma_start(out=dbg["d_oTB"][:, :], in_=oTB[:, :])


# ======================= host-side preparation =========================

def prep_weights(inputs):
    """Fold BN, scale k by 1/sqrt(dh), build all packed weight arrays."""
    f = _as_f32
    bf = ml_dtypes.bfloat16
    qs = f(inputs["q_gamma"]) / np.sqrt(f(inputs["q_var"]) + EPS)
    qb = f(inputs["q_beta"]) - f(inputs["q_mean"]) * qs
    kvs = f(inputs["kv_gamma"]) / np.sqrt(f(inputs["kv_var"]) + EPS)
    kvb = f(inputs["kv_beta"]) - f(inputs["kv_mean"]) * kvs

    dwq = f(inputs["dw_q"])[:, :, 0, :] * qs          # [3,3,C]
    dwkv = f(inputs["dw_kv"])[:, :, 0, :] * kvs
    dwq_t = dwq.reshape(9, C).T.copy()                # [C, 9]
    dwkv_t = dwkv.reshape(9, C).T.copy()

    b_q = qb @ f(inputs["pw_q"])
    b_kv = kvb @ f(inputs["pw_kv"])
    pw_kv = f(inputs["pw_kv"]).copy()
    pw_kv[:, :192] *= SCALE
    b_k = b_kv[:192] * SCALE
    b_v = b_kv[192:]

    def diag_pack(wt, c0, csz):
        m = np.zeros((csz, 9 * csz), np.float32)
        for t in range(9):
            m[np.arange(csz), t * csz + np.arange(csz)] = wt[c0:c0 + csz, t]
        return m.astype(bf)

    pwkv2 = np.zeros((65, 384), np.float32)
    pwkv2[0:64] = pw_kv[128:192]
    pwkv2[64, 192:384] = b_v

    waug2 = np.zeros((65, 192), np.float32)
    waug2[0:64] = f(inputs["out_w"])[128:192]
    waug2[64] = f(inputs["out_b"])

    return {
        "dwq1": dwq_t[0:128].copy(), "dwq2": dwq_t[128:192].copy(),
        "dwkv1c": dwkv_t[0:128].copy(), "dwkv2c": dwkv_t[128:192].copy(),
        "qd1": diag_pack(dwq_t, 0, 128), "qd2": diag_pack(dwq_t, 128, 64),
        "kvd1": diag_pack(dwkv_t, 0, 128), "kvd2": diag_pack(dwkv_t, 128, 64),
        "pwq1": f(inputs["pw_q"])[0:128].astype(bf),
        "pwq2": f(inputs["pw_q"])[128:192].astype(bf),
        "pwkv1": pw_kv[0:128].astype(bf),
        "pwkv2": pwkv2.astype(bf),
        "waug1": f(inputs["out_w"])[0:128].astype(bf),
        "waug2": waug2.astype(bf),
        "bias1": np.stack([b_q[0:128], b_k[0:128]], axis=1).copy(),
        "bias2": np.stack([b_q[128:192], b_k[128:192]], axis=1).copy(),
    }


def prep_x(x):
    """[B,56,56,192] f32 -> list of per-sample padded transposed bf16."""
    bf = ml_dtypes.bfloat16
    x = _as_f32(x)
    xt = np.zeros((B, C, HP, HP), bf)
    xt[:, :, 1:57, 1:57] = x.transpose(0, 3, 1, 2).astype(bf)
    return [xt[b].reshape(C, NPAD) for b in range(B)]


_CACHED_NC = None


def _run(inputs, trace=False, **kwargs):
    global _CACHED_NC
    from concourse.bass_utils import run_bass_kernel_spmd

    if _CACHED_NC is None:
        _CACHED_NC = build_nc()
    nc = _CACHED_NC

    w = prep_weights(inputs)
    xs = prep_x(inputs["x"])
    in_maps = [dict(w, xtp=xs[b]) for b in range(B)]
    res = run_bass_kernel_spmd(nc, in_maps, list(range(B)), trace=trace, **kwargs)
    out = np.stack([np.asarray(res.results[b]["out"], np.float32) for b in range(B)])
    return out.reshape(B, H, W, C), res


def kernel(**inputs):
    return _run(inputs)[0]
